# revision 3
# baseline (speedup 1.0000x reference)
"""ContraNorm Trainium2 kernel v6: out = 1.2*x - 0.2 * softmax(xn @ xn^T) @ x.

Full input x [8, 2048, 512] f32; batch sharded across 8 NeuronCores
(data-parallel, no collectives). Per core: N=2048 rows, D=512.

v6 = v5's software-pipelined full-E dataflow + the row-norm-in-exp-bias trick
that eliminates the fp8 xe cast pass entirely:

  - xf scaled 1.2x in place right after DMA.
  - rn = 1/||row|| via fast-inverse-sqrt on DVE; ln(||row||) via ACT Ln
    (the natural_log_exp_and_others table serves BOTH Ln and Exp, so ACT
    still never swaps tables).
  - xn = xf * rn in fp8 with rn itself stored in column 512 (where the
    "ones" column used to be), PE-transposed into xnT for MM1.
  - MM1 + exp with per-partition bias ln(nrm_r):   E~[p, m] = nrm_p*exp(S).
  - MM2 consumes xn DIRECTLY as rhs: po_d = sum_m nrm_m*exp(S)*xn[m,d]
    = 1.2*(exp(S)@x)  (nrm*rn == 1 cancels the row normalization), and the
    rn column gives po[768] = sum_m exp(S) = D exactly. No xe tensor, no
    cast pass, and any per-row bias error cancels in po/D.
  - finals: ONE scalar_tensor_tensor: ob = po * (-1/(6*D)) + xf.
  - PSUM: "mm" ring (2 bufs x 2 banks) for MM1 halves + transpose scratch;
    "po" pool (2 bufs x 2 banks) for MM2.

Emission is software-pipelined across repeats: within rep k's MM1+exp loop
we interleave rep k+1's phase A and rep k-1's phase C so every engine queue
stays dense; steady-state cost approaches the max per-engine total.
"""

import sys

if "/opt/trn_rl_repo" not in sys.path:
    sys.path.insert(0, "/opt/trn_rl_repo")

from contextlib import ExitStack

import numpy as np

import concourse.bass as bass
import concourse.tile as tile
import concourse.mybir as mybir
from concourse import bacc
from concourse.masks import make_identity
from concourse.bass_utils import run_bass_kernel_spmd

F32 = mybir.dt.float32
FP8 = mybir.dt.float8e4
I32 = mybir.dt.int32
AF = mybir.ActivationFunctionType
ALU = mybir.AluOpType
DR = mybir.MatmulPerfMode.DoubleRow

B = 8
P = 128
N = 2048
D = 512
NT = N // P      # 16 row tiles
DS = D // P      # 4 d subtiles
XNW = 516        # xn row stride (513 used: D cols + rn col at D)
MAGIC = 0x5F3759DF

# engine-split knobs: which instances run on DVE (rest on gpsimd/Pool)
XN_DVE = set()                  # xn scale+cast, 16 tiles
X12_DVE = {0, 1}                # 1.2x in-place batches, 4
CP_DVE = {0, 1, 2, 3}           # xnT psum->sbuf copies (psum: DVE-only)
STT_DVE = set(range(16))        # final combines (psum: DVE-only)


class Pools:
    def __init__(self, ctx: ExitStack, tc: tile.TileContext):
        nc = tc.nc
        self.perm = ctx.enter_context(tc.tile_pool(name="perm", bufs=1))
        self.big = ctx.enter_context(tc.tile_pool(name="big", bufs=2))
        self.stats = ctx.enter_context(tc.tile_pool(name="stats", bufs=2))
        self.obuf = ctx.enter_context(tc.tile_pool(name="obuf", bufs=3))
        self.psMM = ctx.enter_context(tc.tile_pool(name="psMM", bufs=2, space="PSUM"))
        self.psPO = ctx.enter_context(tc.tile_pool(name="psPO", bufs=2, space="PSUM"))
        self.ident = self.perm.tile([P, P], FP8)
        make_identity(nc, self.ident)


class Rep:
    """Per-repeat SBUF state (buffers rotate via pool tags)."""

    def __init__(self, pl: Pools):
        self.xf = pl.big.tile([P, NT, D], F32, tag="xf", bufs=3, name="xf")
        self.xn = pl.big.tile([P, NT, XNW], FP8, tag="xn", bufs=3, name="xn")
        self.xnT = pl.big.tile([P, DS, N], FP8, tag="xnT", name="xnT")
        self.Esb = pl.big.tile([P, NT, N], FP8, tag="Esb", name="Esb")
        self.mv = pl.stats.tile([P, NT, 2], F32, tag="mv", name="mv")
        self.ssq = pl.stats.tile([P, NT], F32, tag="ssq", name="ssq")
        self.rn = pl.stats.tile([P, NT], F32, tag="rn", name="rn")
        self.lnn = pl.stats.tile([P, NT], F32, tag="lnn", name="lnn")


def emit_dma(nc, pl, st, x_ap, t):
    nc.sync.dma_start(st.xf[:, t, :], x_ap[t * P:(t + 1) * P, :])


def emit_x12(nc, pl, st, b):
    # xf *= 1.2 in place, one op per 4 tiles
    x4 = st.xf[:, 4 * b:4 * b + 4, :]
    eng = nc.vector if b in X12_DVE else nc.gpsimd
    eng.tensor_scalar_mul(x4, x4, 1.2)


def emit_stats(nc, pl, st, t):
    bst = pl.stats.tile([P, nc.vector.BN_STATS_DIM], F32, tag="bst", name="bst")
    nc.vector.bn_stats(bst, st.xf[:, t, :])
    nc.vector.bn_aggr(st.mv[:, t, :], bst)


def emit_rsqrt(nc, pl, st, hb):
    """rn = 1/||row||, lnn = ln||row||, and the xn rn-column, for 8 tiles."""
    sl = slice(8 * hb, 8 * hb + 8)
    mv, ssq, rn = st.mv, st.ssq, st.rn
    nc.vector.tensor_tensor(ssq[:, sl], mv[:, sl, 0], mv[:, sl, 0], op=ALU.mult)
    nc.vector.tensor_add(ssq[:, sl], ssq[:, sl], mv[:, sl, 1])
    nc.vector.tensor_scalar_mul(ssq[:, sl], ssq[:, sl], float(D))
    # fast inverse sqrt (bit hack) + 1 Newton step; norms >> eps so the
    # F.normalize clamp is a no-op, and the ~1e-3 error is below fp8 noise.
    ib = pl.stats.tile([P, 8], I32, tag="ib", name="ib")
    nc.vector.tensor_single_scalar(ib, ssq[:, sl].bitcast(I32), 1,
                                   op=ALU.logical_shift_right)
    # (ib - magic) * -1 = magic - (i >> 1)
    nc.vector.tensor_scalar(ib, ib, MAGIC, -1, op0=ALU.subtract, op1=ALU.mult)
    yf = ib.bitcast(F32)
    t2 = pl.stats.tile([P, 8], F32, tag="t2", name="t2")
    nc.vector.tensor_tensor(t2, yf, yf, op=ALU.mult)
    nc.vector.tensor_tensor(t2, t2, ssq[:, sl], op=ALU.mult)
    nc.vector.tensor_scalar(t2, t2, -0.5, 1.5, op0=ALU.mult, op1=ALU.add)
    nc.vector.tensor_tensor(rn[:, sl], yf, t2, op=ALU.mult)
    # lnn = 0.5*ln(ssq) = ln(nrm) via the float-exponent approximation
    # ln(s) ~= ln2*(bits(s)/2^23 - 127.043): +-1% on exp(lnn), far below fp8
    # noise, and entirely on DVE so ACT keeps its Exp table loaded forever.
    lt = pl.stats.tile([P, 8], F32, tag="lt", name="lt")
    nc.vector.tensor_copy(lt, ssq[:, sl].bitcast(I32))  # int value -> f32
    nc.vector.tensor_scalar(st.lnn[:, sl], lt, 0.5 * 0.6931471805599453 / (1 << 23),
                            -0.5 * 0.6931471805599453 * 127.0430357,
                            op0=ALU.mult, op1=ALU.add)
    # rn column of xn (replaces the ones column): makes MM2's col 768 = D
    nc.vector.tensor_copy(st.xn[:, sl, D], rn[:, sl])


def emit_xn(nc, pl, st, t):
    eng = nc.vector if t in XN_DVE else nc.gpsimd
    eng.tensor_scalar_mul(st.xn[:, t, 0:D], st.xf[:, t, :], st.rn[:, t:t + 1])


def emit_ts(nc, pl, st, q):
    """PE-transpose xn tiles 4q..4q+3 into psum, then copy into xnT."""
    # fp8 PE-transpose mode requires output element step 2: write every
    # other byte of the psum scratch and read it back with the same stride.
    ts = pl.psPO.tile([P, 16, 2 * P], FP8, tag="po", name="ts")
    tsv = ts.rearrange("p s (c two) -> p s c two", two=2)[:, :, :, 0]
    for l in range(4):
        for dc in range(DS):
            nc.tensor.transpose(tsv[:, 4 * l + dc, :],
                                st.xn[:, 4 * q + l, dc * P:(dc + 1) * P],
                                pl.ident)
    dst = st.xnT[:, :, 4 * q * P:(4 * q + 4) * P].rearrange(
        "p d (l c) -> p d l c", l=4)
    src = tsv.rearrange("p (l d) c -> p d l c", l=4)
    eng = nc.vector if q in CP_DVE else nc.gpsimd
    eng.tensor_copy(dst, src)


def emit_mm1(nc, pl, st, r):
    """MM1 for row-block r into two fresh psum halves; exp'd one step later
    (keeps next-step MM1 ahead of ts/MM2 in PE's in-order queue)."""
    pss = []
    for h in range(2):
        ps = pl.psMM.tile([P, 1024], F32, tag="mm", name="ps")
        for mb in range(4):
            c0 = h * 1024 + mb * 256
            for g in range(DS // 2):
                nc.tensor.matmul(
                    ps[:, mb * 256:(mb + 1) * 256],
                    lhsT=st.xnT[:, 2 * g:2 * g + 2, r * P:(r + 1) * P],
                    rhs=st.xnT[:, 2 * g:2 * g + 2, c0:c0 + 256],
                    start=(g == 0), stop=(g == 1),
                    perf_mode=DR,
                )
        pss.append(ps)
    st.pending = (r, pss)


def emit_exp(nc, pl, st):
    r, pss = st.pending
    for h in range(2):
        # E~[p, m] = exp(S + ln nrm_p) = nrm_p * exp(S)
        nc.scalar.activation(st.Esb[:, r, h * 1024:(h + 1) * 1024], pss[h],
                             AF.Exp, bias=st.lnn[:, r:r + 1])


def emit_mm2_finals(nc, pl, st, out_ap, i):
    # po layout [128, 1024] (2 banks): d-cols 0:255 at 0 (bank 0), d-cols
    # 256:512 + denominator at 512:769 (bank 1) -- a matmul output must not
    # cross a psum bank boundary.
    po = pl.psPO.tile([P, 1024], F32, tag="po", name="po")
    for g in range(NT // 2):
        lhsT = st.Esb[:, 2 * g:2 * g + 2, i * P:(i + 1) * P]
        nc.tensor.matmul(po[:, 0:256], lhsT, st.xn[:, 2 * g:2 * g + 2, 0:256],
                         start=(g == 0), stop=(g == NT // 2 - 1), perf_mode=DR)
        nc.tensor.matmul(po[:, 512:769], lhsT, st.xn[:, 2 * g:2 * g + 2, 256:513],
                         start=(g == 0), stop=(g == NT // 2 - 1), perf_mode=DR)
    # sD = -1/(6*D)  (po[768] = D = sum_m exp(S))
    mD = pl.stats.tile([P, 1], F32, tag="mD", name="mD")
    nc.vector.tensor_scalar_mul(mD, po[:, 768:769], -6.0)
    sD = pl.stats.tile([P, 1], F32, tag="sD", name="sD")
    nc.vector.reciprocal(sD, mD)
    ob = pl.obuf.tile([P, D], F32, tag="ob", name="ob")
    pov = po.rearrange("p (b c) -> p b c", b=2)[:, :, 0:256]
    obv = ob.rearrange("p (b c) -> p b c", b=2)
    xfv = st.xf[:, i, :].rearrange("p (b c) -> p b c", b=2)
    eng = nc.vector if i in STT_DVE else nc.gpsimd
    eng.scalar_tensor_tensor(obv, pov, sD, xfv, op0=ALU.mult, op1=ALU.add)
    nc.sync.dma_start(out_ap[i * P:(i + 1) * P, :], ob)


# phase-A schedule across the 16 BC steps of the PREVIOUS rep:
#   DMAs 3/step, x12 batches after their DMAs, stats 3/step after x12,
#   rsqrt+ln per 8-tile half, xn 2/step, transposes as soon as inputs ready.
A_DMA = {0: (0, 1, 2), 1: (3, 4, 5), 2: (6, 7, 8), 3: (9, 10, 11),
         4: (12, 13, 14), 5: (15,)}
A_X12 = {1: 0, 2: 1, 3: 2, 5: 3}
A_STATS = {2: (0, 1, 2), 3: (3, 4, 5), 4: (6, 7, 8), 5: (9, 10, 11),
           6: (12, 13, 14), 7: (15,)}
A_RSQ = {4: 0, 7: 1}
A_XN = {5: (0, 1), 6: (2, 3), 7: (4, 5), 8: (6, 7), 9: (8, 9), 10: (10, 11),
        11: (12, 13), 12: (14, 15)}
A_TS = {6: 0, 8: 1, 10: 2, 12: 3}


def emit_A_interleave(nc, pl, st, x_ap, j):
    for t in A_DMA.get(j, ()):
        emit_dma(nc, pl, st, x_ap, t)
    if j in A_X12:
        emit_x12(nc, pl, st, A_X12[j])
    for t in A_STATS.get(j, ()):
        emit_stats(nc, pl, st, t)
    if j in A_RSQ:
        emit_rsqrt(nc, pl, st, A_RSQ[j])
    for t in A_XN.get(j, ()):
        emit_xn(nc, pl, st, t)
    if j in A_TS:
        emit_ts(nc, pl, st, A_TS[j])


def emit_A_prologue(nc, pl, st, x_ap):
    """Standalone phase A for rep 0."""
    for t in range(NT):
        emit_dma(nc, pl, st, x_ap, t)
        if t % 4 == 3:
            emit_x12(nc, pl, st, t // 4)
            for tt in range(t - 3, t + 1):
                emit_stats(nc, pl, st, tt)
        if t == 7:
            emit_rsqrt(nc, pl, st, 0)
        if t == 15:
            emit_rsqrt(nc, pl, st, 1)
    for q in range(4):
        for l in range(4):
            emit_xn(nc, pl, st, 4 * q + l)
        emit_ts(nc, pl, st, q)


def build_nc(repeats: int = 1, loop: int = 0):
    assert loop == 0, "hardware loop mode not supported by the pipelined emitter"
    nc = bacc.Bacc("TRN2", target_bir_lowering=False, debug=False, enable_asserts=False)
    x = nc.dram_tensor("x", [N, D], F32, kind="ExternalInput").ap()
    out = nc.dram_tensor("out", [N, D], F32, kind="ExternalOutput").ap()
    with tile.TileContext(nc) as tc:
        with ExitStack() as ctx:
            pl = Pools(ctx, tc)
            cur = Rep(pl)
            emit_A_prologue(nc, pl, cur, x)
            prev = None
            for k in range(repeats):
                nxt = Rep(pl) if k < repeats - 1 else None
                emit_mm1(nc, pl, cur, 0)
                for j in range(NT):
                    emit_exp(nc, pl, cur)
                    if prev is not None:
                        emit_mm2_finals(nc, pl, prev, out, j)
                    if nxt is not None:
                        emit_A_interleave(nc, pl, nxt, x, j)
                    if j < NT - 1:
                        emit_mm1(nc, pl, cur, j + 1)
                prev, cur = cur, nxt
            # epilogue: phase C of the last rep
            for j in range(NT):
                emit_mm2_finals(nc, pl, prev, out, j)
    nc.compile()
    return nc


_nc_cache = {}


def kernel(x: np.ndarray) -> np.ndarray:
    assert x.shape == (B, N, D), x.shape
    x = np.ascontiguousarray(x, dtype=np.float32)
    if "nc" not in _nc_cache:
        _nc_cache["nc"] = build_nc()
    nc = _nc_cache["nc"]
    in_maps = [{"x": x[i]} for i in range(B)]
    res = run_bass_kernel_spmd(nc, in_maps, core_ids=list(range(B)))
    return np.stack([r["out"] for r in res.results], axis=0)


# revision 4
# speedup vs baseline: 2.6956x; 2.6956x over previous
"""ContraNorm Trainium2 kernel v6: out = 1.2*x - 0.2 * softmax(xn @ xn^T) @ x.

Full input x [8, 2048, 512] f32; batch sharded across 8 NeuronCores
(data-parallel, no collectives). Per core: N=2048 rows, D=512.

v6 = v5's software-pipelined full-E dataflow + the row-norm-in-exp-bias trick
that eliminates the fp8 xe cast pass entirely:

  - xf scaled 1.2x in place right after DMA.
  - rn = 1/||row|| via fast-inverse-sqrt on DVE; ln(||row||) via ACT Ln
    (the natural_log_exp_and_others table serves BOTH Ln and Exp, so ACT
    still never swaps tables).
  - xn = xf * rn in fp8 with rn itself stored in column 512 (where the
    "ones" column used to be), PE-transposed into xnT for MM1.
  - MM1 + exp with per-partition bias ln(nrm_r):   E~[p, m] = nrm_p*exp(S).
  - MM2 consumes xn DIRECTLY as rhs: po_d = sum_m nrm_m*exp(S)*xn[m,d]
    = 1.2*(exp(S)@x)  (nrm*rn == 1 cancels the row normalization), and the
    rn column gives po[768] = sum_m exp(S) = D exactly. No xe tensor, no
    cast pass, and any per-row bias error cancels in po/D.
  - finals: ONE scalar_tensor_tensor: ob = po * (-1/(6*D)) + xf.
  - PSUM: "mm" ring (2 bufs x 2 banks) for MM1 halves + transpose scratch;
    "po" pool (2 bufs x 2 banks) for MM2.

Emission is software-pipelined across repeats: within rep k's MM1+exp loop
we interleave rep k+1's phase A and rep k-1's phase C so every engine queue
stays dense; steady-state cost approaches the max per-engine total.
"""

import sys

if "/opt/trn_rl_repo" not in sys.path:
    sys.path.insert(0, "/opt/trn_rl_repo")

from contextlib import ExitStack

import numpy as np

import concourse.bass as bass
import concourse.tile as tile
import concourse.mybir as mybir
from concourse import bacc
from concourse.masks import make_identity
from concourse.bass_utils import run_bass_kernel_spmd

F32 = mybir.dt.float32
FP8 = mybir.dt.float8e4
I32 = mybir.dt.int32
AF = mybir.ActivationFunctionType
ALU = mybir.AluOpType
DR = mybir.MatmulPerfMode.DoubleRow

B = 8
P = 128
N = 2048
D = 512
NT = N // P      # 16 row tiles
DS = D // P      # 4 d subtiles
XNW = 516        # xn row stride (513 used: D cols + rn col at D)
MAGIC = 0x5F3759DF

# engine-split knobs. GPSIMD/Pool tensor ops measure ~10-40x slower than the
# cost model on real HW (Q7 software), so bulk elementwise lives on DVE/ACT.
XN_DVE = set(range(0, 8))       # xn scale+cast: DVE instances (rest ACT Copy)
X12_DVE = {0, 1, 2, 3}          # 1.2x in-place batches, 4
CP_DVE = {0, 1, 2, 3}           # xnT psum->sbuf copies (psum: DVE-only)
STT_DVE = set(range(16))        # final combines (psum: DVE-only)


class Pools:
    def __init__(self, ctx: ExitStack, tc: tile.TileContext):
        nc = tc.nc
        self.perm = ctx.enter_context(tc.tile_pool(name="perm", bufs=1))
        self.big = ctx.enter_context(tc.tile_pool(name="big", bufs=2))
        self.stats = ctx.enter_context(tc.tile_pool(name="stats", bufs=2))
        self.obuf = ctx.enter_context(tc.tile_pool(name="obuf", bufs=3))
        self.psMM = ctx.enter_context(tc.tile_pool(name="psMM", bufs=2, space="PSUM"))
        self.psPO = ctx.enter_context(tc.tile_pool(name="psPO", bufs=2, space="PSUM"))
        self.ident = self.perm.tile([P, P], FP8)
        make_identity(nc, self.ident)


class Rep:
    """Per-repeat SBUF state (buffers rotate via pool tags)."""

    def __init__(self, pl: Pools):
        self.xf = pl.big.tile([P, NT, D], F32, tag="xf", bufs=3, name="xf")
        self.xn = pl.big.tile([P, NT, XNW], FP8, tag="xn", bufs=3, name="xn")
        self.xnT = pl.big.tile([P, DS, N], FP8, tag="xnT", name="xnT")
        self.Esb = pl.big.tile([P, NT, N], FP8, tag="Esb", name="Esb")
        self.mv = pl.stats.tile([P, NT, 2], F32, tag="mv", name="mv")
        self.ssq = pl.stats.tile([P, NT], F32, tag="ssq", name="ssq")
        self.rn = pl.stats.tile([P, NT], F32, tag="rn", name="rn")
        self.lnn = pl.stats.tile([P, NT], F32, tag="lnn", name="lnn")


def emit_dma(nc, pl, st, x_ap, t):
    nc.sync.dma_start(st.xf[:, t, :], x_ap[t * P:(t + 1) * P, :])


def emit_x12(nc, pl, st, b):
    # xf *= 1.2 in place, one op per 4 tiles
    x4 = st.xf[:, 4 * b:4 * b + 4, :]
    eng = nc.vector if b in X12_DVE else nc.gpsimd
    eng.tensor_scalar_mul(x4, x4, 1.2)  # X12_DVE covers all batches (Pool slow)


def emit_stats(nc, pl, st, t):
    bst = pl.stats.tile([P, nc.vector.BN_STATS_DIM], F32, tag="bst", name="bst")
    nc.vector.bn_stats(bst, st.xf[:, t, :])
    nc.vector.bn_aggr(st.mv[:, t, :], bst)


def emit_rsqrt(nc, pl, st, hb):
    """rn = 1/||row||, lnn = ln||row||, and the xn rn-column, for 8 tiles."""
    sl = slice(8 * hb, 8 * hb + 8)
    mv, ssq, rn = st.mv, st.ssq, st.rn
    nc.vector.tensor_tensor(ssq[:, sl], mv[:, sl, 0], mv[:, sl, 0], op=ALU.mult)
    nc.vector.tensor_add(ssq[:, sl], ssq[:, sl], mv[:, sl, 1])
    nc.vector.tensor_scalar_mul(ssq[:, sl], ssq[:, sl], float(D))
    # fast inverse sqrt (bit hack) + 1 Newton step; norms >> eps so the
    # F.normalize clamp is a no-op, and the ~1e-3 error is below fp8 noise.
    ib = pl.stats.tile([P, 8], I32, tag="ib", name="ib")
    nc.vector.tensor_single_scalar(ib, ssq[:, sl].bitcast(I32), 1,
                                   op=ALU.logical_shift_right)
    # (ib - magic) * -1 = magic - (i >> 1)
    nc.vector.tensor_scalar(ib, ib, MAGIC, -1, op0=ALU.subtract, op1=ALU.mult)
    yf = ib.bitcast(F32)
    t2 = pl.stats.tile([P, 8], F32, tag="t2", name="t2")
    nc.vector.tensor_tensor(t2, yf, yf, op=ALU.mult)
    nc.vector.tensor_tensor(t2, t2, ssq[:, sl], op=ALU.mult)
    nc.vector.tensor_scalar(t2, t2, -0.5, 1.5, op0=ALU.mult, op1=ALU.add)
    nc.vector.tensor_tensor(rn[:, sl], yf, t2, op=ALU.mult)
    # lnn = 0.5*ln(ssq) = ln(nrm) via the float-exponent approximation
    # ln(s) ~= ln2*(bits(s)/2^23 - 127.043): +-1% on exp(lnn), far below fp8
    # noise, and entirely on DVE so ACT keeps its Exp table loaded forever.
    lt = pl.stats.tile([P, 8], F32, tag="lt", name="lt")
    nc.vector.tensor_copy(lt, ssq[:, sl].bitcast(I32))  # int value -> f32
    nc.vector.tensor_scalar(st.lnn[:, sl], lt, 0.5 * 0.6931471805599453 / (1 << 23),
                            -0.5 * 0.6931471805599453 * 127.0430357,
                            op0=ALU.mult, op1=ALU.add)
    # rn column of xn (replaces the ones column): makes MM2's col 768 = D
    nc.vector.tensor_copy(st.xn[:, sl, D], rn[:, sl])


def emit_xn(nc, pl, st, t):
    if t in XN_DVE:
        nc.vector.tensor_scalar_mul(st.xn[:, t, 0:D], st.xf[:, t, :],
                                    st.rn[:, t:t + 1])
    else:
        # ACT Copy with per-partition scale; Copy is in every ACT table so
        # this never causes an activation-table swap.
        nc.scalar.activation(st.xn[:, t, 0:D], st.xf[:, t, :], AF.Copy,
                             scale=st.rn[:, t:t + 1])


def emit_ts(nc, pl, st, q):
    """PE-transpose xn tiles 4q..4q+3 into psum, then copy into xnT."""
    # fp8 PE-transpose mode requires output element step 2: write every
    # other byte of the psum scratch and read it back with the same stride.
    ts = pl.psPO.tile([P, 16, 2 * P], FP8, tag="po", name="ts")
    tsv = ts.rearrange("p s (c two) -> p s c two", two=2)[:, :, :, 0]
    for l in range(4):
        for dc in range(DS):
            nc.tensor.transpose(tsv[:, 4 * l + dc, :],
                                st.xn[:, 4 * q + l, dc * P:(dc + 1) * P],
                                pl.ident)
    dst = st.xnT[:, :, 4 * q * P:(4 * q + 4) * P].rearrange(
        "p d (l c) -> p d l c", l=4)
    src = tsv.rearrange("p (l d) c -> p d l c", l=4)
    eng = nc.vector if q in CP_DVE else nc.gpsimd
    eng.tensor_copy(dst, src)


def emit_mm1(nc, pl, st, r):
    """MM1 for row-block r into two fresh psum halves; exp'd one step later
    (keeps next-step MM1 ahead of ts/MM2 in PE's in-order queue)."""
    pss = []
    for h in range(2):
        ps = pl.psMM.tile([P, 1024], F32, tag="mm", name="ps")
        for mb in range(4):
            c0 = h * 1024 + mb * 256
            for g in range(DS // 2):
                nc.tensor.matmul(
                    ps[:, mb * 256:(mb + 1) * 256],
                    lhsT=st.xnT[:, 2 * g:2 * g + 2, r * P:(r + 1) * P],
                    rhs=st.xnT[:, 2 * g:2 * g + 2, c0:c0 + 256],
                    start=(g == 0), stop=(g == 1),
                    perf_mode=DR,
                )
        pss.append(ps)
    st.pending = (r, pss)


def emit_exp(nc, pl, st):
    r, pss = st.pending
    for h in range(2):
        # E~[p, m] = exp(S + ln nrm_p) = nrm_p * exp(S)
        nc.scalar.activation(st.Esb[:, r, h * 1024:(h + 1) * 1024], pss[h],
                             AF.Exp, bias=st.lnn[:, r:r + 1])


def emit_mm2_finals(nc, pl, st, out_ap, i):
    # po layout [128, 1024] (2 banks): d-cols 0:255 at 0 (bank 0), d-cols
    # 256:512 + denominator at 512:769 (bank 1) -- a matmul output must not
    # cross a psum bank boundary.
    po = pl.psPO.tile([P, 1024], F32, tag="po", name="po")
    for g in range(NT // 2):
        lhsT = st.Esb[:, 2 * g:2 * g + 2, i * P:(i + 1) * P]
        nc.tensor.matmul(po[:, 0:256], lhsT, st.xn[:, 2 * g:2 * g + 2, 0:256],
                         start=(g == 0), stop=(g == NT // 2 - 1), perf_mode=DR)
        nc.tensor.matmul(po[:, 512:769], lhsT, st.xn[:, 2 * g:2 * g + 2, 256:513],
                         start=(g == 0), stop=(g == NT // 2 - 1), perf_mode=DR)
    # sD = -1/(6*D)  (po[768] = D = sum_m exp(S))
    mD = pl.stats.tile([P, 1], F32, tag="mD", name="mD")
    nc.vector.tensor_scalar_mul(mD, po[:, 768:769], -6.0)
    sD = pl.stats.tile([P, 1], F32, tag="sD", name="sD")
    nc.vector.reciprocal(sD, mD)
    ob = pl.obuf.tile([P, D], F32, tag="ob", name="ob")
    pov = po.rearrange("p (b c) -> p b c", b=2)[:, :, 0:256]
    obv = ob.rearrange("p (b c) -> p b c", b=2)
    xfv = st.xf[:, i, :].rearrange("p (b c) -> p b c", b=2)
    eng = nc.vector if i in STT_DVE else nc.gpsimd
    eng.scalar_tensor_tensor(obv, pov, sD, xfv, op0=ALU.mult, op1=ALU.add)
    nc.sync.dma_start(out_ap[i * P:(i + 1) * P, :], ob)


# phase-A schedule across the 16 BC steps of the PREVIOUS rep:
#   DMAs 3/step, x12 batches after their DMAs, stats 3/step after x12,
#   rsqrt+ln per 8-tile half, xn 2/step, transposes as soon as inputs ready.
A_DMA = {0: (0, 1, 2), 1: (3, 4, 5), 2: (6, 7, 8), 3: (9, 10, 11),
         4: (12, 13, 14), 5: (15,)}
A_X12 = {1: 0, 2: 1, 3: 2, 5: 3}
A_STATS = {2: (0, 1, 2), 3: (3, 4, 5), 4: (6, 7, 8), 5: (9, 10, 11),
           6: (12, 13, 14), 7: (15,)}
A_RSQ = {4: 0, 7: 1}
A_XN = {5: (0, 1), 6: (2, 3), 7: (4, 5), 8: (6, 7), 9: (8, 9), 10: (10, 11),
        11: (12, 13), 12: (14, 15)}
A_TS = {6: 0, 8: 1, 10: 2, 12: 3}


def emit_A_interleave(nc, pl, st, x_ap, j):
    for t in A_DMA.get(j, ()):
        emit_dma(nc, pl, st, x_ap, t)
    if j in A_X12:
        emit_x12(nc, pl, st, A_X12[j])
    for t in A_STATS.get(j, ()):
        emit_stats(nc, pl, st, t)
    if j in A_RSQ:
        emit_rsqrt(nc, pl, st, A_RSQ[j])
    for t in A_XN.get(j, ()):
        emit_xn(nc, pl, st, t)
    if j in A_TS:
        emit_ts(nc, pl, st, A_TS[j])


def emit_A_prologue(nc, pl, st, x_ap):
    """Standalone phase A for rep 0."""
    for t in range(NT):
        emit_dma(nc, pl, st, x_ap, t)
        if t % 4 == 3:
            emit_x12(nc, pl, st, t // 4)
            for tt in range(t - 3, t + 1):
                emit_stats(nc, pl, st, tt)
        if t == 7:
            emit_rsqrt(nc, pl, st, 0)
        if t == 15:
            emit_rsqrt(nc, pl, st, 1)
    for q in range(4):
        for l in range(4):
            emit_xn(nc, pl, st, 4 * q + l)
        emit_ts(nc, pl, st, q)


def build_nc(repeats: int = 1, loop: int = 0):
    assert loop == 0, "hardware loop mode not supported by the pipelined emitter"
    nc = bacc.Bacc("TRN2", target_bir_lowering=False, debug=False, enable_asserts=False)
    x = nc.dram_tensor("x", [N, D], F32, kind="ExternalInput").ap()
    out = nc.dram_tensor("out", [N, D], F32, kind="ExternalOutput").ap()
    with tile.TileContext(nc) as tc:
        with ExitStack() as ctx:
            pl = Pools(ctx, tc)
            cur = Rep(pl)
            emit_A_prologue(nc, pl, cur, x)
            prev = None
            for k in range(repeats):
                nxt = Rep(pl) if k < repeats - 1 else None
                emit_mm1(nc, pl, cur, 0)
                for j in range(NT):
                    emit_exp(nc, pl, cur)
                    if prev is not None:
                        emit_mm2_finals(nc, pl, prev, out, j)
                    if nxt is not None:
                        emit_A_interleave(nc, pl, nxt, x, j)
                    if j < NT - 1:
                        emit_mm1(nc, pl, cur, j + 1)
                prev, cur = cur, nxt
            # epilogue: phase C of the last rep
            for j in range(NT):
                emit_mm2_finals(nc, pl, prev, out, j)
    nc.compile()
    return nc


_nc_cache = {}


def kernel(x: np.ndarray) -> np.ndarray:
    assert x.shape == (B, N, D), x.shape
    x = np.ascontiguousarray(x, dtype=np.float32)
    if "nc" not in _nc_cache:
        _nc_cache["nc"] = build_nc()
    nc = _nc_cache["nc"]
    in_maps = [{"x": x[i]} for i in range(B)]
    res = run_bass_kernel_spmd(nc, in_maps, core_ids=list(range(B)))
    return np.stack([r["out"] for r in res.results], axis=0)


# revision 5
# speedup vs baseline: 3.0530x; 1.1326x over previous
"""ContraNorm Trainium2 kernel: out = 1.2*x - 0.2 * softmax(xn @ xn^T) @ x.

Full input x [8, 2048, 512] f32; batch sharded across 8 NeuronCores
(data-parallel, no collectives). Per core: N=2048 rows, D=512.

Key facts this design is built on (measured on the axon TRN2 path):
  - GPSIMD/Pool tensor ops run ~10-40x slower than the concourse cost model
    (Q7 software); ALL bulk elementwise lives on DVE or ACT. GPSIMD also
    cannot access PSUM at all (walrus verifier rejects it).
  - DVE gets ~2x throughput on SBUF-only operand sets (2x_2p mode).
  - ACT must never swap activation tables: it runs Exp only (plus Copy,
    which is in every table). Sqrt/Ln are done on DVE via bit tricks.
  - fp8 PE-transpose mode requires output element step 2 in PSUM.

Dataflow per repeat (software-pipelined emission across repeats: rep k's
MM1+exp loop interleaves rep k+1's phase A and rep k-1's phase C):
  - xf scaled 1.2x in place right after DMA (DVE): stats/xn self-normalize
    and the final combine's +xf term IS the 1.2*x term.
  - rn = 1/||row|| via fast-inverse-sqrt (bitcast + Newton) on DVE;
    ln||row|| via the float-exponent approximation on DVE (error cancels in
    the softmax ratio).
  - xn = xf * rn in fp8 (DVE tensor_scalar / ACT Copy with scale), with rn
    itself stored in column 512; PE-transposed (stride-2 fp8) into xnT,
    psum->sbuf copies on DVE.
  - MM1 row-block r, half h: 8 fp8 DoubleRow matmuls -> [128,1024] PSUM;
    ONE ACT exp with per-partition bias ln(nrm_r): E~ = nrm_r * exp(S).
  - E~ is symmetric up to row scales; E~[:, 2g:2g+2, i*128:...] serves as
    MM2's lhsT, and MM2's rhs is xn directly: the nrm_m * rn_m product
    cancels the normalization, po[768] = D exactly via the rn column.
  - finals: ONE DVE scalar_tensor_tensor: ob = po * (-1/(6*D)) + xf.
  - PSUM: "mm" ring (2 bufs x 2 banks) for MM1; "po" pool (2 bufs x 2
    banks) for MM2 + transpose scratch.
"""

import sys

if "/opt/trn_rl_repo" not in sys.path:
    sys.path.insert(0, "/opt/trn_rl_repo")

from contextlib import ExitStack

import numpy as np

import concourse.bass as bass
import concourse.tile as tile
import concourse.mybir as mybir
from concourse import bacc
from concourse.masks import make_identity
from concourse.bass_utils import run_bass_kernel_spmd

F32 = mybir.dt.float32
FP8 = mybir.dt.float8e4
I32 = mybir.dt.int32
AF = mybir.ActivationFunctionType
ALU = mybir.AluOpType
DR = mybir.MatmulPerfMode.DoubleRow

B = 8
P = 128
N = 2048
D = 512
NT = N // P      # 16 row tiles
DS = D // P      # 4 d subtiles
XNW = 516        # xn row stride (513 used: D cols + rn col at D)
MAGIC = 0x5F3759DF

# engine-split knobs. GPSIMD/Pool tensor ops measure ~10-40x slower than the
# cost model on real HW (Q7 software), so bulk elementwise lives on DVE/ACT.
XN_DVE = set(range(0, 8))       # xn scale+cast: DVE instances (rest ACT Copy)
X12_DVE = {0, 1, 2, 3}          # 1.2x in-place batches, 4
CP_DVE = {0, 1, 2, 3}           # xnT psum->sbuf copies (psum: DVE-only)
STT_DVE = set(range(16))        # final combines (psum: DVE-only)


class Pools:
    def __init__(self, ctx: ExitStack, tc: tile.TileContext):
        nc = tc.nc
        self.perm = ctx.enter_context(tc.tile_pool(name="perm", bufs=1))
        self.big = ctx.enter_context(tc.tile_pool(name="big", bufs=2))
        self.stats = ctx.enter_context(tc.tile_pool(name="stats", bufs=2))
        self.obuf = ctx.enter_context(tc.tile_pool(name="obuf", bufs=3))
        self.psMM = ctx.enter_context(tc.tile_pool(name="psMM", bufs=2, space="PSUM"))
        self.psPO = ctx.enter_context(tc.tile_pool(name="psPO", bufs=2, space="PSUM"))
        self.ident = self.perm.tile([P, P], FP8)
        make_identity(nc, self.ident)


class Rep:
    """Per-repeat SBUF state (buffers rotate via pool tags)."""

    def __init__(self, pl: Pools):
        self.xf = pl.big.tile([P, NT, D], F32, tag="xf", bufs=3, name="xf")
        self.xn = pl.big.tile([P, NT, XNW], FP8, tag="xn", bufs=3, name="xn")
        self.xnT = pl.big.tile([P, DS, N], FP8, tag="xnT", name="xnT")
        self.Esb = pl.big.tile([P, NT, N], FP8, tag="Esb", name="Esb")
        self.mv = pl.stats.tile([P, NT, 2], F32, tag="mv", name="mv")
        self.ssq = pl.stats.tile([P, NT], F32, tag="ssq", name="ssq")
        self.rn = pl.stats.tile([P, NT], F32, tag="rn", name="rn")
        self.lnn = pl.stats.tile([P, NT], F32, tag="lnn", name="lnn")


def emit_dma(nc, pl, st, x_ap, t):
    nc.sync.dma_start(st.xf[:, t, :], x_ap[t * P:(t + 1) * P, :])


def emit_x12(nc, pl, st, b):
    # xf *= 1.2 in place, one op per 4 tiles
    x4 = st.xf[:, 4 * b:4 * b + 4, :]
    eng = nc.vector if b in X12_DVE else nc.gpsimd
    eng.tensor_scalar_mul(x4, x4, 1.2)  # X12_DVE covers all batches (Pool slow)


def emit_stats(nc, pl, st, t):
    bst = pl.stats.tile([P, nc.vector.BN_STATS_DIM], F32, tag="bst", name="bst")
    nc.vector.bn_stats(bst, st.xf[:, t, :])
    nc.vector.bn_aggr(st.mv[:, t, :], bst)


def emit_rsqrt(nc, pl, st, hb):
    """rn = 1/||row||, lnn = ln||row||, and the xn rn-column, for 8 tiles."""
    sl = slice(8 * hb, 8 * hb + 8)
    mv, ssq, rn = st.mv, st.ssq, st.rn
    nc.vector.tensor_tensor(ssq[:, sl], mv[:, sl, 0], mv[:, sl, 0], op=ALU.mult)
    nc.vector.tensor_add(ssq[:, sl], ssq[:, sl], mv[:, sl, 1])
    nc.vector.tensor_scalar_mul(ssq[:, sl], ssq[:, sl], float(D))
    # fast inverse sqrt (bit hack) + 1 Newton step; norms >> eps so the
    # F.normalize clamp is a no-op, and the ~1e-3 error is below fp8 noise.
    ib = pl.stats.tile([P, 8], I32, tag="ib", name="ib")
    nc.vector.tensor_single_scalar(ib, ssq[:, sl].bitcast(I32), 1,
                                   op=ALU.logical_shift_right)
    # (ib - magic) * -1 = magic - (i >> 1)
    nc.vector.tensor_scalar(ib, ib, MAGIC, -1, op0=ALU.subtract, op1=ALU.mult)
    yf = ib.bitcast(F32)
    t2 = pl.stats.tile([P, 8], F32, tag="t2", name="t2")
    nc.vector.tensor_tensor(t2, yf, yf, op=ALU.mult)
    nc.vector.tensor_tensor(t2, t2, ssq[:, sl], op=ALU.mult)
    nc.vector.tensor_scalar(t2, t2, -0.5, 1.5, op0=ALU.mult, op1=ALU.add)
    nc.vector.tensor_tensor(rn[:, sl], yf, t2, op=ALU.mult)
    # lnn = 0.5*ln(ssq) = ln(nrm) via the float-exponent approximation
    # ln(s) ~= ln2*(bits(s)/2^23 - 127.043): +-1% on exp(lnn), far below fp8
    # noise, and entirely on DVE so ACT keeps its Exp table loaded forever.
    lt = pl.stats.tile([P, 8], F32, tag="lt", name="lt")
    nc.vector.tensor_copy(lt, ssq[:, sl].bitcast(I32))  # int value -> f32
    nc.vector.tensor_scalar(st.lnn[:, sl], lt, 0.5 * 0.6931471805599453 / (1 << 23),
                            -0.5 * 0.6931471805599453 * 127.0430357,
                            op0=ALU.mult, op1=ALU.add)
    # rn column of xn (replaces the ones column): makes MM2's col 768 = D
    nc.vector.tensor_copy(st.xn[:, sl, D], rn[:, sl])


def emit_xn(nc, pl, st, t):
    if t in XN_DVE:
        nc.vector.tensor_scalar_mul(st.xn[:, t, 0:D], st.xf[:, t, :],
                                    st.rn[:, t:t + 1])
    else:
        # ACT Copy with per-partition scale; Copy is in every ACT table so
        # this never causes an activation-table swap.
        nc.scalar.activation(st.xn[:, t, 0:D], st.xf[:, t, :], AF.Copy,
                             scale=st.rn[:, t:t + 1])


def emit_ts(nc, pl, st, q):
    """PE-transpose xn tiles 4q..4q+3 into psum, then copy into xnT."""
    # fp8 PE-transpose mode requires output element step 2: write every
    # other byte of the psum scratch and read it back with the same stride.
    ts = pl.psPO.tile([P, 16, 2 * P], FP8, tag="po", name="ts")
    tsv = ts.rearrange("p s (c two) -> p s c two", two=2)[:, :, :, 0]
    for l in range(4):
        for dc in range(DS):
            nc.tensor.transpose(tsv[:, 4 * l + dc, :],
                                st.xn[:, 4 * q + l, dc * P:(dc + 1) * P],
                                pl.ident)
    dst = st.xnT[:, :, 4 * q * P:(4 * q + 4) * P].rearrange(
        "p d (l c) -> p d l c", l=4)
    src = tsv.rearrange("p (l d) c -> p d l c", l=4)
    eng = nc.vector if q in CP_DVE else nc.gpsimd
    eng.tensor_copy(dst, src)


def emit_mm1(nc, pl, st, r):
    """MM1 for row-block r into two fresh psum halves; exp'd one step later
    (keeps next-step MM1 ahead of ts/MM2 in PE's in-order queue)."""
    pss = []
    for h in range(2):
        ps = pl.psMM.tile([P, 1024], F32, tag="mm", name="ps")
        for mb in range(4):
            c0 = h * 1024 + mb * 256
            for g in range(DS // 2):
                nc.tensor.matmul(
                    ps[:, mb * 256:(mb + 1) * 256],
                    lhsT=st.xnT[:, 2 * g:2 * g + 2, r * P:(r + 1) * P],
                    rhs=st.xnT[:, 2 * g:2 * g + 2, c0:c0 + 256],
                    start=(g == 0), stop=(g == 1),
                    perf_mode=DR,
                )
        pss.append(ps)
    st.pending = (r, pss)


def emit_exp(nc, pl, st):
    r, pss = st.pending
    for h in range(2):
        # E~[p, m] = exp(S + ln nrm_p) = nrm_p * exp(S)
        nc.scalar.activation(st.Esb[:, r, h * 1024:(h + 1) * 1024], pss[h],
                             AF.Exp, bias=st.lnn[:, r:r + 1])


def emit_mm2_finals(nc, pl, st, out_ap, i):
    # po layout [128, 1024] (2 banks): d-cols 0:255 at 0 (bank 0), d-cols
    # 256:512 + denominator at 512:769 (bank 1) -- a matmul output must not
    # cross a psum bank boundary.
    po = pl.psPO.tile([P, 1024], F32, tag="po", name="po")
    for g in range(NT // 2):
        lhsT = st.Esb[:, 2 * g:2 * g + 2, i * P:(i + 1) * P]
        nc.tensor.matmul(po[:, 0:256], lhsT, st.xn[:, 2 * g:2 * g + 2, 0:256],
                         start=(g == 0), stop=(g == NT // 2 - 1), perf_mode=DR)
        nc.tensor.matmul(po[:, 512:769], lhsT, st.xn[:, 2 * g:2 * g + 2, 256:513],
                         start=(g == 0), stop=(g == NT // 2 - 1), perf_mode=DR)
    # sD = -1/(6*D)  (po[768] = D = sum_m exp(S))
    mD = pl.stats.tile([P, 1], F32, tag="mD", name="mD")
    nc.vector.tensor_scalar_mul(mD, po[:, 768:769], -6.0)
    sD = pl.stats.tile([P, 1], F32, tag="sD", name="sD")
    nc.vector.reciprocal(sD, mD)
    ob = pl.obuf.tile([P, D], F32, tag="ob", name="ob")
    pov = po.rearrange("p (b c) -> p b c", b=2)[:, :, 0:256]
    obv = ob.rearrange("p (b c) -> p b c", b=2)
    xfv = st.xf[:, i, :].rearrange("p (b c) -> p b c", b=2)
    eng = nc.vector if i in STT_DVE else nc.gpsimd
    eng.scalar_tensor_tensor(obv, pov, sD, xfv, op0=ALU.mult, op1=ALU.add)
    nc.sync.dma_start(out_ap[i * P:(i + 1) * P, :], ob)


# phase-A schedule across the 16 BC steps of the PREVIOUS rep:
#   DMAs 3/step, x12 batches after their DMAs, stats 3/step after x12,
#   rsqrt+ln per 8-tile half, xn 2/step, transposes as soon as inputs ready.
A_DMA = {0: (0, 1, 2), 1: (3, 4, 5), 2: (6, 7, 8), 3: (9, 10, 11),
         4: (12, 13, 14), 5: (15,)}
A_X12 = {1: 0, 2: 1, 3: 2, 5: 3}
A_STATS = {2: (0, 1, 2), 3: (3, 4, 5), 4: (6, 7, 8), 5: (9, 10, 11),
           6: (12, 13, 14), 7: (15,)}
A_RSQ = {4: 0, 7: 1}
A_XN = {5: (0, 1), 6: (2, 3), 7: (4, 5), 8: (6, 7), 9: (8, 9), 10: (10, 11),
        11: (12, 13), 12: (14, 15)}
A_TS = {6: 0, 8: 1, 10: 2, 12: 3}


def emit_A_interleave(nc, pl, st, x_ap, j):
    for t in A_DMA.get(j, ()):
        emit_dma(nc, pl, st, x_ap, t)
    if j in A_X12:
        emit_x12(nc, pl, st, A_X12[j])
    for t in A_STATS.get(j, ()):
        emit_stats(nc, pl, st, t)
    if j in A_RSQ:
        emit_rsqrt(nc, pl, st, A_RSQ[j])
    for t in A_XN.get(j, ()):
        emit_xn(nc, pl, st, t)
    if j in A_TS:
        emit_ts(nc, pl, st, A_TS[j])


def emit_A_prologue(nc, pl, st, x_ap):
    """Standalone phase A for rep 0."""
    for t in range(NT):
        emit_dma(nc, pl, st, x_ap, t)
        if t % 4 == 3:
            emit_x12(nc, pl, st, t // 4)
            for tt in range(t - 3, t + 1):
                emit_stats(nc, pl, st, tt)
        if t == 7:
            emit_rsqrt(nc, pl, st, 0)
        if t == 15:
            emit_rsqrt(nc, pl, st, 1)
    for q in range(4):
        for l in range(4):
            emit_xn(nc, pl, st, 4 * q + l)
        emit_ts(nc, pl, st, q)


def build_nc(repeats: int = 1, loop: int = 0):
    assert loop == 0, "hardware loop mode not supported by the pipelined emitter"
    nc = bacc.Bacc("TRN2", target_bir_lowering=False, debug=False, enable_asserts=False)
    x = nc.dram_tensor("x", [N, D], F32, kind="ExternalInput").ap()
    out = nc.dram_tensor("out", [N, D], F32, kind="ExternalOutput").ap()
    with tile.TileContext(nc) as tc:
        with ExitStack() as ctx:
            pl = Pools(ctx, tc)
            cur = Rep(pl)
            emit_A_prologue(nc, pl, cur, x)
            prev = None
            for k in range(repeats):
                nxt = Rep(pl) if k < repeats - 1 else None
                emit_mm1(nc, pl, cur, 0)
                for j in range(NT):
                    emit_exp(nc, pl, cur)
                    if prev is not None:
                        emit_mm2_finals(nc, pl, prev, out, j)
                    if nxt is not None:
                        emit_A_interleave(nc, pl, nxt, x, j)
                    if j < NT - 1:
                        emit_mm1(nc, pl, cur, j + 1)
                prev, cur = cur, nxt
            # epilogue: phase C of the last rep
            for j in range(NT):
                emit_mm2_finals(nc, pl, prev, out, j)
    nc.compile()
    return nc


_nc_cache = {}


def kernel(x: np.ndarray) -> np.ndarray:
    assert x.shape == (B, N, D), x.shape
    x = np.ascontiguousarray(x, dtype=np.float32)
    if "nc" not in _nc_cache:
        _nc_cache["nc"] = build_nc()
    nc = _nc_cache["nc"]
    in_maps = [{"x": x[i]} for i in range(B)]
    res = run_bass_kernel_spmd(nc, in_maps, core_ids=list(range(B)))
    return np.stack([r["out"] for r in res.results], axis=0)
